# revision 39
# baseline (speedup 1.0000x reference)
"""Trainium2 Bass kernel for nn_CapsuleLayer (capsule layer w/ dynamic routing).

Math (reference):
    u_hat[b,c,u,s] = sum_p W[c,u,s,p] * X[b,p,c]
    b_ij = 0
    3x: c_ij = softmax_c(b_ij); s_j = sum_c c_ij*u_hat; v = squash_u(s_j)
        b_ij += mean_b sum_s u_hat*v
    return v[..., None]

Strategy: shard C=1152 across 8 cores (144 each); never materialize u_hat.
The per-core contraction index is the flattened pair r=(c_local*8+p), 1152
rows = 9 partition-tiles of 128. All operands are laid out host-side in
numpy (bf16): XT[r,b], WT[r,(u,s)], XZ[b,r] — no on-device transposes or
permutes, so setup is DMA-only. Per iteration (e = exp(b_ij)):
    A[r,(u,s)]   = WT * e_r            (DVE, e broadcast from PSUM)
    s_raw[b,us] += XT_t.T @ A_t        (9 PE matmuls, PSUM-accumulated)
    dp[u]        = sum_c e             (PE ones-matmul + tiny reduce)
    collective (AllReduce; ReduceScatter on the last iteration so each
        core only squashes its own 16 batch rows)
    s_j = s_raw/denom ; v = s_j * mag/(1+mag_sq)       (squash)
    Z_t[r,(u,s)] = XZ_t.T @ v          (9 PE matmuls -> PSUM)
    p1 = WT * Z ; p1r[r,(t,u)] = sum_s p1              (DVE)
    agr16[j,(t,u)] = G.T @ p1r         (PE matmul sums the 8 p's per c)
    e16 *= exp(agr16/B)  (ACT) ;  e_r = H.T @ e16      (PE broadcast)
The agree partition-reduction and the e row-broadcast both run on the PE
via constant 0/1 matrices G[128,16] / H[16,128], keeping the DVE chain
short. Collective payload rows are [row(160) | denom(10) | pad(6)] so the
summed denominator arrives per-partition (no broadcast after readback).
"""

import numpy as np
import ml_dtypes

import concourse.bass as bass
import concourse.mybir as mybir
import concourse.tile as tile
from concourse import bacc
from concourse.bass_utils import run_bass_kernel_spmd

B, P, C, U, S = 128, 8, 1152, 10, 16
R = 3
NCORES = 8
CL = C // NCORES          # 144 local capsules
RPC = CL * P              # 1152 local (c,p) rows
NT = RPC // 128           # 9 partition tiles
US = U * S                # 160
TU = NT * U               # 90
ROW = US + S              # 160 payload + 10 denom + 6 pad = 176 (32B-aligned)
BSL = B // NCORES         # 16 batch rows per core in the final scatter
CH = 3                    # tiles per pipeline chunk
F32 = mybir.dt.float32
BF16 = mybir.dt.bfloat16
ADD = mybir.AluOpType.add
MULT = mybir.AluOpType.mult
EXP = mybir.ActivationFunctionType.Exp

BF = ml_dtypes.bfloat16


def _build_program():
    nc = bacc.Bacc("TRN2", target_bir_lowering=False, debug=False,
                   num_devices=NCORES)
    XTl = nc.dram_tensor("XTl", [RPC, B], BF16, kind="ExternalInput")
    WTl = nc.dram_tensor("WTl", [RPC, US], BF16, kind="ExternalInput")
    XZl = nc.dram_tensor("XZl", [B, RPC], BF16, kind="ExternalInput")
    Ml = nc.dram_tensor("Ml", [128, 128], BF16, kind="ExternalInput")
    Vout = nc.dram_tensor("Vout", [BSL, US], F32, kind="ExternalOutput")

    with tile.TileContext(nc) as tc:
        _emit(nc, tc, XTl, WTl, XZl, Ml, Vout)
    nc.compile()
    return nc


def _emit(nc, tc, XTl, WTl, XZl, Ml, Vout):
    rg = [list(range(NCORES))]
    with (
        tc.tile_pool(name="const", bufs=1) as cpool,
        tc.tile_pool(name="work", bufs=2) as wpool,
        tc.tile_pool(name="p1p", bufs=1) as p1pool,
        tc.tile_pool(name="psz", bufs=1, space="PSUM") as zpool,
        tc.tile_pool(name="pss", bufs=1, space="PSUM") as spool,
        tc.tile_pool(name="pse", bufs=1, space="PSUM") as epool,
        tc.tile_pool(name="dram", bufs=2, space="DRAM") as dpool,
    ):
        # ---------------- constants / setup (DMA only) ----------------
        XT = cpool.tile([128, NT * B], BF16, tag="XT")
        WT = cpool.tile([128, NT * US], BF16, tag="WT")
        XZ = cpool.tile([B, RPC], BF16, tag="XZ")
        M = cpool.tile([128, 128], BF16, tag="M")

        def _tiled_src(dram, ncol, t0, t1):
            # dram [(t q), ncol] viewed as [q, t in t0:t1, ncol]
            ap = dram[:]
            return bass.AP(ap.tensor, ap.offset + t0 * 128 * ncol,
                           [[ncol, 128], [128 * ncol, t1 - t0], [1, ncol]])

        # XT/WT gate the t=0 matmuls: interleave across all 3 DMA queues so
        # the first chunks land first
        nc.sync.dma_start(XT[:, 0:3 * B], _tiled_src(XTl, B, 0, 3))
        nc.scalar.dma_start(WT[:, 0:3 * US], _tiled_src(WTl, US, 0, 3))
        nc.gpsimd.dma_start(XT[:, 3 * B:6 * B], _tiled_src(XTl, B, 3, 6))
        nc.sync.dma_start(WT[:, 3 * US:6 * US], _tiled_src(WTl, US, 3, 6))
        nc.scalar.dma_start(XT[:, 6 * B:], _tiled_src(XTl, B, 6, NT))
        nc.gpsimd.dma_start(WT[:, 6 * US:], _tiled_src(WTl, US, 6, NT))
        nc.sync.dma_start(M[:], Ml[:])
        # XZ is first needed after AR0 returns (iteration-0 Z matmuls)
        nc.scalar.dma_start(XZ[:, 0:RPC // 2], XZl[:, 0:RPC // 2])
        nc.gpsimd.dma_start(XZ[:, RPC // 2:], XZl[:, RPC // 2:])

        ones_col = cpool.tile([128, 1], BF16, tag="onescol")
        nc.vector.memset(ones_col[:], 1.0)
        # 1/8: each e[c,u] appears 8x (once per p) in the row-broadcast e_r,
        # so the denominator matmul folds the correction into this lhsT
        ones_row = cpool.tile([1, 128], F32, tag="onesrow")
        nc.vector.memset(ones_row[:], 0.125)
        dp_pad = cpool.tile([1, S], F32, tag="dppad")
        nc.vector.memset(dp_pad[:], 0.0)
        scratch = cpool.tile([1, 4], F32, tag="scratch")
        nc.vector.memset(scratch[:], 1.0)
        # ACT LUT preloads: exp first, sqrt last (sqrt is the first real use)
        nc.scalar.activation(scratch[:, 2:3], scratch[:, 3:4], EXP)
        nc.scalar.sqrt(scratch[:, 0:1], scratch[:, 1:2])

        e_r = None   # exp(b_ij) row-broadcast [128, (t,u)] SBUF bf16; None=>1

        for t in range(R):
            last = t == R - 1

            # ---------- s_raw = sum_t XT_t.T @ (WT_t * e) ----------
            s_ps = spool.tile([B, US], F32, tag="s", name=f"sps{t}")
            if t == 0:
                for tt in range(NT):
                    nc.tensor.matmul(
                        s_ps[:, :], XT[:, tt * B:(tt + 1) * B],
                        WT[:, tt * US:(tt + 1) * US],
                        start=(tt == 0), stop=(tt == NT - 1))
            else:
                A = wpool.tile([128, NT * US], BF16, tag="A", name=f"A{t}")
                for ch in range(CH):
                    t0, t1 = ch * 3, min(ch * 3 + 3, NT)
                    ntt = t1 - t0
                    # gpsimd takes the middle chunk so the DVE only
                    # serializes two chunks (e_r is SBUF: both can read it)
                    eng = nc.gpsimd if ch == 1 else nc.vector
                    eb = bass.AP(
                        e_r[:].tensor, e_r[:].offset + t0 * U,
                        [e_r[:].ap[0], [U, ntt], [1, U], [0, S]])
                    eng.tensor_tensor(
                        A[:, t0 * US:t1 * US].rearrange(
                            "q (a u s) -> q a u s", u=U, s=S),
                        WT[:, t0 * US:t1 * US].rearrange(
                            "q (a u s) -> q a u s", u=U, s=S),
                        eb, MULT)
                    for tt in range(t0, t1):
                        nc.tensor.matmul(
                            s_ps[:, :], XT[:, tt * B:(tt + 1) * B],
                            A[:, tt * US:(tt + 1) * US],
                            start=(tt == 0), stop=(tt == NT - 1))

            # ---------- denominator partials (t>0; t=0 denom == C) --------
            if t > 0:
                dp16 = epool.tile([1, TU], F32, tag="dps", name=f"dp16{t}")
                nc.tensor.matmul(dp16[:, :], ones_col[:], e_r[:],
                                 start=True, stop=True)
                dpu = wpool.tile([1, U], F32, tag="dpu", name=f"dpu{t}")
                dsrc = bass.AP(dp16[:].tensor, dp16[:].offset,
                               [dp16[:].ap[0], [1, U], [U, NT]])
                nc.vector.tensor_reduce(dpu[:], dsrc,
                                        axis=mybir.AxisListType.X, op=ADD)
                nc.gpsimd.tensor_copy(dp_pad[:, 0:U], dpu[:])
                dpb = epool.tile([B, S], F32, tag="dps", name=f"dpb{t}")
                nc.tensor.matmul(dpb[:, :], ones_row[:, :], dp_pad[:, :],
                                 start=True, stop=True)
                dp_rep = wpool.tile([B, S], BF16 if not last else F32,
                                    tag=f"dprep{int(last)}", name=f"dpr{t}")
                nc.vector.tensor_copy(dp_rep[:, :], dpb[:, :])

            # ---------- stage + collective ----------
            rw = ROW if t > 0 else US
            ccn = B * rw
            ccdt = BF16 if not last else F32
            cc_in = dpool.tile([ccn], ccdt, tag=f"ccin{t}")
            cc_out = dpool.tile([ccn // (NCORES if last else 1)], ccdt,
                                tag=f"ccout{t}")
            src = wpool.tile([B, US], BF16 if not last else F32,
                             tag=f"sstage{int(last)}", name=f"sstage{t}")
            nc.vector.tensor_copy(src[:, :], s_ps[:, :])
            nc.sync.dma_start(
                bass.AP(cc_in[:].tensor, cc_in[:].offset, [[rw, 64], [1, US]]),
                src[0:64, :])
            nc.scalar.dma_start(
                bass.AP(cc_in[:].tensor, cc_in[:].offset + 64 * rw,
                        [[rw, 64], [1, US]]),
                src[64:128, :])
            if t > 0:
                nc.gpsimd.dma_start(
                    bass.AP(cc_in[:].tensor, cc_in[:].offset + US,
                            [[ROW, B], [1, S]]),
                    dp_rep[:, :])
            nc.gpsimd.collective_compute(
                "AllReduce" if not last else "ReduceScatter", ADD,
                replica_groups=rg,
                ins=[cc_in[:].opt()], outs=[cc_out[:].opt()])

            nb = B if not last else BSL
            if last:
                s_in = wpool.tile([nb, rw], F32, tag="sinl", name=f"sin{t}")
                nc.gpsimd.dma_start(
                    s_in[:, :],
                    cc_out[:].rearrange("(b f) -> b f", b=nb))
            else:
                s_in = wpool.tile([nb, rw], BF16, tag="sin16",
                                  name=f"sin16_{t}")
                h = nb // 2
                nc.gpsimd.dma_start(
                    s_in[0:h, :],
                    cc_out[0:h * rw].rearrange("(b f) -> b f", b=h))
                nc.sync.dma_start(
                    s_in[h:nb, :],
                    cc_out[h * rw:].rearrange("(b f) -> b f", b=h))
            s_sum = s_in[:, 0:US]

            # ---------- s_j = s_sum / denom ----------
            s_j = wpool.tile([nb, US], F32, tag=f"sj{min(t, 1)}",
                             name=f"sj{t}")
            if t == 0:
                nc.vector.tensor_scalar_mul(s_j[:], s_sum, 1.0 / C)
            else:
                rd = wpool.tile([nb, U], F32, tag=f"rd{min(t, 1)}",
                                name=f"rd{t}")
                nc.vector.reciprocal(rd[:], s_in[:, US:US + U])
                nc.vector.tensor_tensor(
                    s_j[:].rearrange("q (u s) -> q u s", s=S),
                    s_sum.rearrange("q (u s) -> q u s", s=S),
                    rd[:].unsqueeze(2).broadcast_to((nb, U, S)), MULT)

            # ---------- v = squash(s_j): v = s_j * mag/(1+mag_sq) ----------
            sq = wpool.tile([nb, US], F32, tag=f"sq{min(t,1)}", name=f"sq{t}")
            nc.vector.tensor_tensor(sq[:], s_j[:], s_j[:], MULT)
            msq = wpool.tile([nb, S], F32, tag=f"msq{min(t,1)}",
                             name=f"msq{t}")
            nc.vector.tensor_reduce(
                msq[:], sq[:].rearrange("q (u s) -> q s u", u=U),
                axis=mybir.AxisListType.X, op=ADD)
            mag = wpool.tile([nb, S], F32, tag=f"mag{min(t,1)}",
                             name=f"mag{t}")
            nc.scalar.sqrt(mag[:], msq[:])
            if not last:
                # keep the EXP LUT resident for the upcoming e-update
                dex = wpool.tile([1, 1], F32, tag="dex", name=f"dex{t}")
                nc.scalar.activation(dex[:], mag[0:1, 0:1], EXP)
            h1 = wpool.tile([nb, S], F32, tag=f"h1{min(t,1)}", name=f"h1{t}")
            nc.vector.tensor_scalar_add(h1[:], msq[:], 1.0)
            rh = wpool.tile([nb, S], F32, tag=f"rh{min(t,1)}", name=f"rh{t}")
            nc.vector.reciprocal(rh[:], h1[:])
            g = wpool.tile([nb, S], F32, tag=f"g{min(t,1)}", name=f"g{t}")
            nc.vector.tensor_tensor(g[:], mag[:], rh[:], MULT)
            v_sb = wpool.tile([nb, US], F32 if last else BF16,
                              tag=f"v{min(t,1)}", name=f"v{t}")
            nc.vector.tensor_tensor(
                v_sb[:].rearrange("q (u s) -> q u s", s=S),
                s_j[:].rearrange("q (u s) -> q u s", s=S),
                g[:].unsqueeze(1).broadcast_to((nb, U, S)), MULT)

            if last:
                nc.sync.dma_start(Vout[:, :], v_sb[:, :])
                break

            # ---------- Z_t = XZ_t.T @ v ; p1 = WT*Z ; row-sums ----------
            # p1 chunks on DVE; the s-reductions run on gpsimd (SBUF->SBUF)
            # so they overlap the next p1 chunk
            p1 = p1pool.tile([128, NT * US], BF16, tag="p1", name=f"p1_{t}")
            p1r = wpool.tile([128, TU], BF16, tag="p1r", name=f"p1r{t}")
            with nc.allow_low_precision(reason="p1r feeds a f32-psum matmul"):
                for ch in range(CH):
                    t0, t1 = ch * 3, min(ch * 3 + 3, NT)
                    ntt = t1 - t0
                    zt = zpool.tile([128, 512], F32, tag=f"z{ch}",
                                    name=f"z{ch}_{t}")
                    for k in range(ntt):
                        tt = t0 + k
                        nc.tensor.matmul(
                            zt[:, k * US:(k + 1) * US],
                            XZ[:, tt * 128:(tt + 1) * 128], v_sb[:],
                            start=True, stop=True)
                    nc.vector.tensor_tensor(
                        p1[:, t0 * US:t1 * US], WT[:, t0 * US:t1 * US],
                        zt[:, 0:ntt * US], MULT)
                    nc.vector.tensor_reduce(
                        p1r[:, t0 * U:t1 * U].rearrange(
                            "q (a u) -> q a u", u=U),
                        p1[:, t0 * US:t1 * US].rearrange(
                            "q (a u s) -> q a u s", u=U, s=S),
                        axis=mybir.AxisListType.X, op=ADD)

            # ---------- agr_r = M.T @ p1r  (M sums the 8 p's per c AND
            # broadcasts back to rows) ; e_r *= exp(agr_r/B) ----------
            agrb = epool.tile([128, TU], F32, tag="er", name=f"agrb{t}")
            nc.tensor.matmul(agrb[:, :], M[:], p1r[:], start=True, stop=True)
            eg = wpool.tile([128, TU], BF16, tag="eg", name=f"eg{t}")
            nc.scalar.activation(eg[:], agrb[:], EXP, scale=1.0 / B)
            if e_r is None:
                e_r = eg
            else:
                e_n = wpool.tile([128, TU], BF16, tag="ern", name=f"ern{t}")
                nc.vector.tensor_tensor(e_n[:], e_r[:], eg[:], MULT)
                e_r = e_n
            # preload SQRT table for the next squash (after the real exp)
            dsq = wpool.tile([1, 1], F32, tag="dsq", name=f"dsq{t}")
            nc.scalar.sqrt(dsq[:], eg[0:1, 0:1])


_NC_CACHE = None
_M_HOST = None


def _get_program():
    global _NC_CACHE
    if _NC_CACHE is None:
        _NC_CACHE = _build_program()
    return _NC_CACHE


def _m_host():
    global _M_HOST
    if _M_HOST is None:
        r = np.arange(128)
        _M_HOST = (r[:, None] // 8 == r[None, :] // 8).astype(BF)
    return _M_HOST


def _in_maps(X, W):
    m = _m_host()
    maps = []
    for i in range(NCORES):
        sl = slice(i * CL, (i + 1) * CL)
        Xs = np.asarray(X[:, :, sl], dtype=np.float32)   # [B, P, CL]
        Ws = np.asarray(W[sl], dtype=np.float32)         # [CL, U, S, P]
        wt = np.ascontiguousarray(
            Ws.transpose(0, 3, 1, 2)).reshape(RPC, US).astype(BF)
        xt = np.ascontiguousarray(
            Xs.transpose(2, 1, 0)).reshape(RPC, B).astype(BF)
        xz = np.ascontiguousarray(
            Xs.transpose(0, 2, 1)).reshape(B, RPC).astype(BF)
        maps.append({"XTl": xt, "WTl": wt, "XZl": xz, "Ml": m})
    return maps


def kernel(X: np.ndarray, W: np.ndarray) -> np.ndarray:
    assert X.shape == (B, P, C) and W.shape == (C, U, S, P)
    nc = _get_program()
    res = run_bass_kernel_spmd(nc, _in_maps(X, W),
                               core_ids=list(range(NCORES)))
    out = np.empty((B, US), dtype=np.float32)
    for i in range(NCORES):
        out[i * BSL:(i + 1) * BSL] = res.results[i]["Vout"]
    return out.reshape(B, U, S, 1)
